# revision 7
# baseline (speedup 1.0000x reference)
"""Distributed multi-head-attention kernel for 8 TRN2 NeuronCores.

Problem (hardcoded): B=4, T=S=1024, E=512, H=8, head_dim=64, fp32 I/O.
Sharding: core c handles batch b=c//2 and heads [4*(c%2), 4*(c%2)+4).
No collectives: each core produces a partial output projection
(contraction over its 256 ctx columns); the host sums the two partials
per batch and adds bo.

Compute dtype: bf16 on the TensorEngine (fp32 PSUM accumulation),
softmax in fp32 on ScalarE/VectorE.
"""

import numpy as np
import ml_dtypes

import concourse.bass as bass
import concourse.tile as tile
import concourse.mybir as mybir
from concourse.bass_utils import run_bass_kernel_spmd

BF16 = mybir.dt.bfloat16
F32 = mybir.dt.float32
NPBF16 = ml_dtypes.bfloat16

B, T, S, E = 4, 1024, 1024, 512
H, HD = 8, 64
N_CORES = 8
HPC = H // 2          # heads per core = 4
OS = E // 2           # o-slice width per core = 256
KT = E // 128         # contraction k-tiles for projections = 4
TT = T // 128         # token tiles = 8
TC = T // 512         # 512-wide token chunks = 2

# ---------------------------------------------------------------------------
# Walrus in this container rejects instructions carrying more than a couple of
# sync waits. After Tile scheduling, split excess waits onto same-engine NOPs
# inserted immediately before the over-subscribed instruction.
# ---------------------------------------------------------------------------
_MAX_WAITS = 1
_split_ctr = [0]


def _split_sync_waits(nc, max_waits=_MAX_WAITS):
    for f in nc.m.functions:
        for bb in f.blocks:
            insts = bb.instructions
            if not any(i.sync_info and i.sync_info.on_wait
                       and len(i.sync_info.on_wait) > max_waits for i in insts):
                continue
            new = []
            for inst in insts:
                si = inst.sync_info
                if si is not None and si.on_wait and len(si.on_wait) > max_waits:
                    waits = list(si.on_wait)
                    extra, keep = waits[:-max_waits], waits[-max_waits:]
                    for j in range(0, len(extra), max_waits):
                        _split_ctr[0] += 1
                        nop = mybir.InstNoOp(
                            name=f"syncsplit-{_split_ctr[0]}", ins=[], outs=[])
                        nop.engine = inst.engine
                        nop.bass_nofuse = True
                        nop.text_hint = "syncsplit"
                        nop.sync_info = mybir.SyncInfo(
                            on_wait=extra[j:j + max_waits], on_update=[])
                        new.append(nop)
                    si.on_wait = keep
                new.append(inst)
            bb.instructions = new


# ---------------------------------------------------------------------------
# Kernel graph
# ---------------------------------------------------------------------------
def build_nc():
    nc = bass.Bass()

    xqT = nc.declare_dram_parameter("xqT", [E, T], BF16, isOutput=False)
    xkT = nc.declare_dram_parameter("xkT", [E, S], BF16, isOutput=False)
    xvT = nc.declare_dram_parameter("xvT", [E, S], BF16, isOutput=False)
    wqT = nc.declare_dram_parameter("wqT", [E, OS], BF16, isOutput=False)
    wkT = nc.declare_dram_parameter("wkT", [E, OS], BF16, isOutput=False)
    wvT = nc.declare_dram_parameter("wvT", [E, OS], BF16, isOutput=False)
    # per-head slices of Wo^T: [64 (c), 512 (e)] each
    woTs = [nc.declare_dram_parameter(f"woT{h}", [HD, E], BF16, isOutput=False)
            for h in range(HPC)]
    bq_t = nc.declare_dram_parameter("bq_t", [128, 2], F32, isOutput=False)
    bk_t = nc.declare_dram_parameter("bk_t", [128, 2], F32, isOutput=False)
    bv_b = nc.declare_dram_parameter("bv_b", [128, OS], F32, isOutput=False)
    out_ext = nc.declare_dram_parameter("out", [T, E], F32, isOutput=True)

    # DRAM bounces for softmax-denominator reciprocal re-layout + broadcast
    r_dram = nc.dram_tensor("r_bounce", [HPC, T], F32)
    r2_dram = nc.dram_tensor("rinv_bounce", [HPC, T], F32)

    with tile.TileContext(nc) as tc:
        with (
            tc.tile_pool(name="inp", bufs=1) as inp,
            tc.tile_pool(name="wts", bufs=1) as wts,
            tc.tile_pool(name="act", bufs=1) as actp,
            tc.tile_pool(name="et", bufs=3) as etp,
            tc.tile_pool(name="rb", bufs=2) as rbp,
            tc.tile_pool(name="psum", bufs=1, space="PSUM") as psum,
        ):
            # ---- input DMAs -------------------------------------------------
            xq_sb = inp.tile([128, KT, T], BF16)
            nc.sync.dma_start(xq_sb[:], xqT.ap().rearrange("(k p) t -> p k t", p=128))
            xk_sb = inp.tile([128, KT, S], BF16)
            nc.sync.dma_start(xk_sb[:], xkT.ap().rearrange("(k p) t -> p k t", p=128))
            xv_sb = inp.tile([128, KT, S], BF16)
            nc.sync.dma_start(xv_sb[:], xvT.ap().rearrange("(k p) t -> p k t", p=128))

            wq_sb = wts.tile([128, KT, OS], BF16)
            nc.sync.dma_start(wq_sb[:], wqT.ap().rearrange("(k p) o -> p k o", p=128))
            wk_sb = wts.tile([128, KT, OS], BF16)
            nc.sync.dma_start(wk_sb[:], wkT.ap().rearrange("(k p) o -> p k o", p=128))
            wv_sb = wts.tile([128, KT, OS], BF16)
            nc.sync.dma_start(wv_sb[:], wvT.ap().rearrange("(k p) o -> p k o", p=128))
            wo_sbs = []
            for h in range(HPC):
                wo_sb = wts.tile([HD, E], BF16, name=f"wo{h}")
                nc.sync.dma_start(wo_sb[:], woTs[h].ap())
                wo_sbs.append(wo_sb)
            bq_sb = wts.tile([128, 2], F32, name="bq")
            nc.sync.dma_start(bq_sb[:], bq_t.ap())
            bk_sb = wts.tile([128, 2], F32, name="bk")
            nc.sync.dma_start(bk_sb[:], bk_t.ap())
            bv_sb = wts.tile([128, OS], F32, name="bv")
            nc.sync.dma_start(bv_sb[:], bv_b.ap())

            # ---- projections -----------------------------------------------
            # q^T, k^T: [o(128) x t] tiles, 2 o-tiles each (2 heads per tile)
            qT_sb = actp.tile([128, 2, T], BF16, name="qT")
            kT_sb = actp.tile([128, 2, S], BF16, name="kT")
            for (src, w_sb, b_sb, dst) in (
                (xq_sb, wq_sb, bq_sb, qT_sb),
                (xk_sb, wk_sb, bk_sb, kT_sb),
            ):
                for ot in range(2):
                    ps = psum.tile([128, T], F32, name="projq", tag="mmio", bufs=1)
                    for tc_i in range(TC):
                        for k in range(KT):
                            nc.tensor.matmul(
                                ps[:, 512 * tc_i:512 * (tc_i + 1)],
                                w_sb[:, k, 128 * ot:128 * (ot + 1)],
                                src[:, k, 512 * tc_i:512 * (tc_i + 1)],
                                start=(k == 0), stop=(k == KT - 1),
                            )
                    nc.vector.tensor_scalar_add(
                        dst[:, ot, :], ps[:], b_sb[:, ot:ot + 1])

            # v natural layout, augmented with a ones column per head:
            # v_aug[p, st, h, 0:64] = v[s, 64h:64h+64];  v_aug[p, st, h, 64] = 1
            v_aug = actp.tile([128, TT, HPC, HD + 1], BF16, name="vaug")
            nc.gpsimd.memset(v_aug[:, :, :, HD:HD + 1], 1.0)
            for st in range(TT):
                ps = psum.tile([128, OS], F32, name="projv", tag="mmio", bufs=1)
                for k in range(KT):
                    nc.tensor.matmul(
                        ps[:],
                        xv_sb[:, k, 128 * st:128 * (st + 1)],
                        wv_sb[:, k, :],
                        start=(k == 0), stop=(k == KT - 1),
                    )
                nc.vector.tensor_add(
                    v_aug[:, st, :, 0:HD],
                    ps.rearrange("p (h d) -> p h d", h=HPC),
                    bv_sb.rearrange("p (h d) -> p h d", h=HPC),
                )

            # ---- attention per head ----------------------------------------
            ctx_nrm = [actp.tile([HD, T], BF16, name=f"ctx{h}") for h in range(HPC)]
            for h in range(HPC):
                ot, po = h // 2, HD * (h % 2)
                c_ps = psum.tile([HD + 1, T], F32, name="ctxps", tag="ctxps", bufs=1)
                for st in range(TT):
                    s_ps = psum.tile([128, T], F32, name="sps", tag="sps", bufs=2)
                    for tc_i in range(TC):
                        nc.tensor.matmul(
                            s_ps[:, 512 * tc_i:512 * (tc_i + 1)],
                            kT_sb[po:po + HD, ot, 128 * st:128 * (st + 1)],
                            qT_sb[po:po + HD, ot, 512 * tc_i:512 * (tc_i + 1)],
                            start=True, stop=True,
                        )
                    et = etp.tile([128, T], BF16, name="et")
                    nc.scalar.activation(
                        et[:], s_ps[:], mybir.ActivationFunctionType.Exp,
                        scale=float(1.0 / np.sqrt(HD)))
                    for tc_i in range(TC):
                        nc.tensor.matmul(
                            c_ps[:, 512 * tc_i:512 * (tc_i + 1)],
                            v_aug[:, st, h, :],
                            et[:, 512 * tc_i:512 * (tc_i + 1)],
                            start=(st == 0), stop=(st == TT - 1),
                        )
                # softmax denominators: row HD of c_ps holds r[t]. Re-layout
                # r to [128 t, 8] via a DRAM bounce so the exact DVE
                # reciprocal runs on all 128 lanes, scatter 1/r back, then
                # broadcast across partitions with a stride-0 DMA.
                rsb = rbp.tile([HD + 1, T], F32, name="rsb")
                nc.vector.tensor_copy(rsb[HD:HD + 1, :], c_ps[HD:HD + 1, :])
                nc.sync.dma_start(r_dram[h:h + 1, :], rsb[HD:HD + 1, :])
                rT = rbp.tile([128, TT], F32, name="rT")
                nc.sync.dma_start(
                    rT[:], r_dram[h:h + 1, :].rearrange("a (c p) -> p a c", p=128))
                rinvT = rbp.tile([128, TT], F32, name="rinvT")
                nc.vector.reciprocal(rinvT[:], rT[:])
                nc.sync.dma_start(
                    r2_dram[h:h + 1, :].rearrange("a (c p) -> p a c", p=128),
                    rinvT[:])
                rb = rbp.tile([HD, T], F32, name="rb")
                nc.sync.dma_start(
                    rb[:], r2_dram[h:h + 1, :].partition_broadcast(HD))
                nc.vector.tensor_mul(ctx_nrm[h][:], c_ps[0:HD, :], rb[:])

            # ---- output projection -----------------------------------------
            for tt in range(TT):
                o_ps = psum.tile([128, E], F32, name="ops", tag="mmio", bufs=1)
                for h in range(HPC):
                    nc.tensor.matmul(
                        o_ps[:],
                        ctx_nrm[h][:, 128 * tt:128 * (tt + 1)],
                        wo_sbs[h][:],
                        start=(h == 0), stop=(h == HPC - 1),
                    )
                o_sb = rbp.tile([128, E], F32, name="osb")
                nc.vector.tensor_copy(o_sb[:], o_ps[:])
                nc.sync.dma_start(out_ext[128 * tt:128 * (tt + 1), :], o_sb[:])

    _split_sync_waits(nc)
    return nc


_NC = None


def _get_nc():
    global _NC
    if _NC is None:
        _NC = build_nc()
    return _NC


# ---------------------------------------------------------------------------
# Host-side sharding / unsharding
# ---------------------------------------------------------------------------
def make_in_maps(queries, keys, values, Wq, bq, Wk, bk, Wv, bv, Wo):
    in_maps = []
    for c in range(N_CORES):
        b, hh = divmod(c, 2)
        osl = slice(OS * hh, OS * (hh + 1))
        bq_s = np.zeros((128, 2), np.float32)
        bq_s[:, 0] = bq[osl][0:128]
        bq_s[:, 1] = bq[osl][128:256]
        bk_s = np.zeros((128, 2), np.float32)
        bk_s[:, 0] = bk[osl][0:128]
        bk_s[:, 1] = bk[osl][128:256]
        m = {
            "xqT": np.ascontiguousarray(queries[b].T).astype(NPBF16),
            "xkT": np.ascontiguousarray(keys[b].T).astype(NPBF16),
            "xvT": np.ascontiguousarray(values[b].T).astype(NPBF16),
            "wqT": np.ascontiguousarray(Wq[osl, :].T).astype(NPBF16),
            "wkT": np.ascontiguousarray(Wk[osl, :].T).astype(NPBF16),
            "wvT": np.ascontiguousarray(Wv[osl, :].T).astype(NPBF16),
            "bq_t": bq_s,
            "bk_t": bk_s,
            "bv_b": np.broadcast_to(
                bv[osl][None, :], (128, OS)).astype(np.float32).copy(),
        }
        for h in range(HPC):
            cs = slice(OS * hh + HD * h, OS * hh + HD * (h + 1))
            m[f"woT{h}"] = np.ascontiguousarray(Wo[:, cs].T).astype(NPBF16)
        in_maps.append(m)
    return in_maps


def run_device(in_maps, trace=False):
    nc = _get_nc()
    return run_bass_kernel_spmd(
        nc, in_maps, core_ids=list(range(N_CORES)), trace=trace)


def _numpy_reference(queries, keys, values, Wq, bq, Wk, bk, Wv, bv, Wo, bo,
                     q_padding_mask, key_padding_mask, attn_mask):
    q = queries @ Wq.T + bq
    k = keys @ Wk.T + bk
    v = values @ Wv.T + bv

    def split(x):
        b, l, e = x.shape
        return x.reshape(b, l, H, HD).transpose(0, 2, 1, 3)

    q, k, v = split(q), split(k), split(v)
    scores = np.einsum('bhtd,bhsd->bhts', q, k) / np.sqrt(HD)
    scores = np.where(key_padding_mask[:, None, None, :], -np.inf, scores)
    scores = np.where(~attn_mask[None, None, :, :], -np.inf, scores)
    scores = scores - scores.max(axis=-1, keepdims=True)
    w = np.exp(scores)
    w = w / w.sum(axis=-1, keepdims=True)
    w = np.where(q_padding_mask[:, None, :, None], 0.0, w)
    ctx = np.einsum('bhts,bhsd->bhtd', w, v)
    ctx = ctx.transpose(0, 2, 1, 3).reshape(queries.shape[0], -1, E)
    return (ctx @ Wo.T + bo).astype(np.float32)


def kernel(queries, keys, values, Wq, bq, Wk, bk, Wv, bv, Wo, bo,
           q_padding_mask, key_padding_mask, attn_mask):
    queries = np.asarray(queries, dtype=np.float32)
    keys = np.asarray(keys, dtype=np.float32)
    values = np.asarray(values, dtype=np.float32)
    Wq, bq = np.asarray(Wq, np.float32), np.asarray(bq, np.float32)
    Wk, bk = np.asarray(Wk, np.float32), np.asarray(bk, np.float32)
    Wv, bv = np.asarray(Wv, np.float32), np.asarray(bv, np.float32)
    Wo, bo = np.asarray(Wo, np.float32), np.asarray(bo, np.float32)
    q_padding_mask = np.asarray(q_padding_mask)
    key_padding_mask = np.asarray(key_padding_mask)
    attn_mask = np.asarray(attn_mask)

    # The device kernel skips masking (and softmax max-subtraction, valid for
    # this problem's bounded score range). Masks are all-trivial per the
    # problem spec; fall back to a host reference if they ever are not.
    if q_padding_mask.any() or key_padding_mask.any() or not attn_mask.all():
        return _numpy_reference(
            queries, keys, values, Wq, bq, Wk, bk, Wv, bv, Wo, bo,
            q_padding_mask, key_padding_mask, attn_mask)

    in_maps = make_in_maps(queries, keys, values, Wq, bq, Wk, bk, Wv, bv, Wo)
    res = run_device(in_maps, trace=False)
    out = np.empty((B, T, E), np.float32)
    for b in range(B):
        out[b] = (res.results[2 * b]["out"] + res.results[2 * b + 1]["out"]
                  + bo[None, :])
    return out


# revision 9
# speedup vs baseline: 1.4417x; 1.4417x over previous
"""Distributed multi-head-attention kernel for 8 TRN2 NeuronCores.

Problem (hardcoded): B=4, T=S=1024, E=512, H=8, head_dim=64, fp32 I/O.
Sharding: core c handles batch b=c//2 and heads [4*(c%2), 4*(c%2)+4).
No collectives: each core produces a partial output projection
(contraction over its 256 ctx columns); the host sums the two partials
per batch and adds bo.

Compute dtype: bf16 on the TensorEngine (fp32 PSUM accumulation),
softmax in fp32 on ScalarE/VectorE.
"""

import numpy as np
import ml_dtypes

import concourse.bass as bass
import concourse.tile as tile
import concourse.mybir as mybir
from concourse.bass_utils import run_bass_kernel_spmd

BF16 = mybir.dt.bfloat16
F32 = mybir.dt.float32
NPBF16 = ml_dtypes.bfloat16

B, T, S, E = 4, 1024, 1024, 512
H, HD = 8, 64
N_CORES = 8
HPC = H // 2          # heads per core = 4
OS = E // 2           # o-slice width per core = 256
KT = E // 128         # contraction k-tiles for projections = 4
TT = T // 128         # token tiles = 8
TC = T // 512         # 512-wide token chunks = 2

# ---------------------------------------------------------------------------
# Walrus in this container rejects instructions carrying more than a couple of
# sync waits. After Tile scheduling, split excess waits onto same-engine NOPs
# inserted immediately before the over-subscribed instruction.
# ---------------------------------------------------------------------------
_MAX_WAITS = 1
_split_ctr = [0]


def _split_sync_waits(nc, max_waits=_MAX_WAITS):
    for f in nc.m.functions:
        for bb in f.blocks:
            insts = bb.instructions
            if not any(i.sync_info and i.sync_info.on_wait
                       and len(i.sync_info.on_wait) > max_waits for i in insts):
                continue
            new = []
            for inst in insts:
                si = inst.sync_info
                if si is not None and si.on_wait and len(si.on_wait) > max_waits:
                    waits = list(si.on_wait)
                    extra, keep = waits[:-max_waits], waits[-max_waits:]
                    for j in range(0, len(extra), max_waits):
                        _split_ctr[0] += 1
                        nop = mybir.InstNoOp(
                            name=f"syncsplit-{_split_ctr[0]}", ins=[], outs=[])
                        nop.engine = inst.engine
                        nop.bass_nofuse = True
                        nop.text_hint = "syncsplit"
                        nop.sync_info = mybir.SyncInfo(
                            on_wait=extra[j:j + max_waits], on_update=[])
                        new.append(nop)
                    si.on_wait = keep
                new.append(inst)
            bb.instructions = new


# ---------------------------------------------------------------------------
# Kernel graph
# ---------------------------------------------------------------------------
def build_nc():
    nc = bass.Bass()

    xqT = nc.declare_dram_parameter("xqT", [E, T], BF16, isOutput=False)
    xkT = nc.declare_dram_parameter("xkT", [E, S], BF16, isOutput=False)
    xvT = nc.declare_dram_parameter("xvT", [E, S], BF16, isOutput=False)
    wqT = nc.declare_dram_parameter("wqT", [E, OS], BF16, isOutput=False)
    wkT = nc.declare_dram_parameter("wkT", [E, OS], BF16, isOutput=False)
    wvT = nc.declare_dram_parameter("wvT", [E, OS], BF16, isOutput=False)
    # per-head slices of Wo^T: [64 (c), 512 (e)] each
    woTs = [nc.declare_dram_parameter(f"woT{h}", [HD, E], BF16, isOutput=False)
            for h in range(HPC)]
    bq_t = nc.declare_dram_parameter("bq_t", [128, 2], F32, isOutput=False)
    bk_t = nc.declare_dram_parameter("bk_t", [128, 2], F32, isOutput=False)
    bv_b = nc.declare_dram_parameter("bv_b", [128, OS], F32, isOutput=False)
    ident = nc.declare_dram_parameter("ident", [128, 128], F32, isOutput=False)
    out_ext = nc.declare_dram_parameter("out", [T, E], F32, isOutput=True)

    # DRAM bounce for softmax-denominator reciprocal broadcast
    r2_dram = nc.dram_tensor("rinv_bounce", [HPC, TT, 128], F32)

    with tile.TileContext(nc) as tc:
        with (
            tc.tile_pool(name="inp", bufs=1) as inp,
            tc.tile_pool(name="wts", bufs=1) as wts,
            tc.tile_pool(name="act", bufs=1) as actp,
            tc.tile_pool(name="et", bufs=4) as etp,
            tc.tile_pool(name="rb", bufs=2) as rbp,
            tc.tile_pool(name="psum", bufs=1, space="PSUM") as psum,
        ):
            # ---- input DMAs (weights first; inputs split per k-tile) -------
            wq_sb = wts.tile([128, KT, OS], BF16)
            nc.sync.dma_start(wq_sb[:], wqT.ap().rearrange("(k p) o -> p k o", p=128))
            wk_sb = wts.tile([128, KT, OS], BF16)
            nc.sync.dma_start(wk_sb[:], wkT.ap().rearrange("(k p) o -> p k o", p=128))
            wv_sb = wts.tile([128, KT, OS], BF16)
            nc.sync.dma_start(wv_sb[:], wvT.ap().rearrange("(k p) o -> p k o", p=128))
            wo_sbs = []
            for h in range(HPC):
                wo_sb = wts.tile([HD, E], BF16, name=f"wo{h}")
                nc.sync.dma_start(wo_sb[:], woTs[h].ap())
                wo_sbs.append(wo_sb)
            bq_sb = wts.tile([128, 2], F32, name="bq")
            nc.sync.dma_start(bq_sb[:], bq_t.ap())
            bk_sb = wts.tile([128, 2], F32, name="bk")
            nc.sync.dma_start(bk_sb[:], bk_t.ap())
            bv_sb = wts.tile([128, OS], F32, name="bv")
            nc.sync.dma_start(bv_sb[:], bv_b.ap())
            id_sb = wts.tile([128, 128], F32, name="ident")
            nc.sync.dma_start(id_sb[:], ident.ap())

            xq_sb = inp.tile([128, KT, T], BF16)
            xk_sb = inp.tile([128, KT, S], BF16)
            xv_sb = inp.tile([128, KT, S], BF16)
            for (dst, srcp) in ((xq_sb, xqT), (xk_sb, xkT), (xv_sb, xvT)):
                rr = srcp.ap().rearrange("(k p) t -> p k t", p=128)
                for k in range(KT):
                    nc.sync.dma_start(dst[:, k:k + 1, :], rr[:, k:k + 1, :])

            # ---- projections -----------------------------------------------
            # q^T, k^T: [o(128) x t] tiles, 2 o-tiles each (2 heads per tile)
            qT_sb = actp.tile([128, 2, T], BF16, name="qT")
            kT_sb = actp.tile([128, 2, S], BF16, name="kT")
            for (src, w_sb, b_sb, dst) in (
                (xq_sb, wq_sb, bq_sb, qT_sb),
                (xk_sb, wk_sb, bk_sb, kT_sb),
            ):
                for ot in range(2):
                    ps = psum.tile([128, T], F32, name="projq", tag="sps", bufs=2)
                    for tc_i in range(TC):
                        for k in range(KT):
                            nc.tensor.matmul(
                                ps[:, 512 * tc_i:512 * (tc_i + 1)],
                                w_sb[:, k, 128 * ot:128 * (ot + 1)],
                                src[:, k, 512 * tc_i:512 * (tc_i + 1)],
                                start=(k == 0), stop=(k == KT - 1),
                            )
                    nc.vector.tensor_scalar_add(
                        dst[:, ot, :], ps[:], b_sb[:, ot:ot + 1])

            # v natural layout, augmented with a ones column per head:
            # v_aug[p, st, h, 0:64] = v[s, 64h:64h+64];  v_aug[p, st, h, 64] = 1
            v_aug = actp.tile([128, TT, HPC, HD + 1], BF16, name="vaug")
            nc.gpsimd.memset(v_aug[:, :, :, HD:HD + 1], 1.0)
            for st in range(TT):
                ps = psum.tile([128, OS], F32, name="projv", tag="sps", bufs=2)
                for k in range(KT):
                    nc.tensor.matmul(
                        ps[:],
                        xv_sb[:, k, 128 * st:128 * (st + 1)],
                        wv_sb[:, k, :],
                        start=(k == 0), stop=(k == KT - 1),
                    )
                nc.vector.tensor_add(
                    v_aug[:, st, :, 0:HD],
                    ps.rearrange("p (h d) -> p h d", h=HPC),
                    bv_sb.rearrange("p (h d) -> p h d", h=HPC),
                )

            # ---- attention per head ----------------------------------------
            ctx_nrm = [actp.tile([HD, T], BF16, name=f"ctx{h}") for h in range(HPC)]
            rsb = actp.tile([128, T], F32, name="rsb")
            nc.gpsimd.memset(rsb[:], 0.0)
            for h in range(HPC):
                ot, po = h // 2, HD * (h % 2)
                c_ps = psum.tile([HD + 1, T], F32, name="ctxps", tag="ctxps", bufs=2)
                for st in range(TT):
                    s_ps = psum.tile([128, T], F32, name="sps", tag="sps", bufs=2)
                    for tc_i in range(TC):
                        nc.tensor.matmul(
                            s_ps[:, 512 * tc_i:512 * (tc_i + 1)],
                            kT_sb[po:po + HD, ot, 128 * st:128 * (st + 1)],
                            qT_sb[po:po + HD, ot, 512 * tc_i:512 * (tc_i + 1)],
                            start=True, stop=True,
                        )
                    et = etp.tile([128, T], BF16, name="et")
                    nc.scalar.activation(
                        et[:], s_ps[:], mybir.ActivationFunctionType.Exp,
                        scale=float(1.0 / np.sqrt(HD)))
                    for tc_i in range(TC):
                        nc.tensor.matmul(
                            c_ps[:, 512 * tc_i:512 * (tc_i + 1)],
                            v_aug[:, st, h, :],
                            et[:, 512 * tc_i:512 * (tc_i + 1)],
                            start=(st == 0), stop=(st == TT - 1),
                        )
                # softmax denominators: row HD of c_ps holds r[t].
                # PE-transpose r into [t(part), chunk] layout, run the exact
                # DVE reciprocal on all 128 lanes, transpose back, then
                # broadcast across partitions with a stride-0 DMA via DRAM.
                nc.vector.tensor_copy(rsb[HD:HD + 1, :], c_ps[HD:HD + 1, :])
                tp_ps = psum.tile([128, T], F32, name="tps", tag="sps", bufs=2)
                for c in range(TT):
                    nc.tensor.matmul(
                        tp_ps[:, 128 * c:128 * (c + 1)],
                        rsb[:, 128 * c:128 * (c + 1)], id_sb[:],
                        is_transpose=True, start=True, stop=True)
                rT = rbp.tile([128, TT], F32, name="rT")
                nc.vector.tensor_copy(
                    rT[:],
                    tp_ps.rearrange("p (c x) -> p c x", x=128)[:, :, HD])
                rinvT = rbp.tile([128, TT], F32, name="rinvT")
                nc.vector.reciprocal(rinvT[:], rT[:])
                nc.tensor.matmul(
                    tp_ps[0:TT, 0:128], rinvT[:], id_sb[:],
                    is_transpose=True, start=True, stop=True)
                r8 = rbp.tile([TT, 128], F32, name="r8")
                nc.vector.tensor_copy(r8[:], tp_ps[0:TT, 0:128])
                nc.sync.dma_start(r2_dram[h], r8[:])
                rb = rbp.tile([HD, T], F32, name="rb")
                nc.sync.dma_start(
                    rb[:],
                    r2_dram.ap().rearrange("h c p -> h (c p)")[h:h + 1, :]
                    .partition_broadcast(HD))
                nc.vector.tensor_mul(ctx_nrm[h][:], c_ps[0:HD, :], rb[:])

            # ---- output projection -----------------------------------------
            for tt in range(TT):
                o_ps = psum.tile([128, E], F32, name="ops", tag="sps", bufs=2)
                for h in range(HPC):
                    nc.tensor.matmul(
                        o_ps[:],
                        ctx_nrm[h][:, 128 * tt:128 * (tt + 1)],
                        wo_sbs[h][:],
                        start=(h == 0), stop=(h == HPC - 1),
                    )
                o_sb = rbp.tile([128, E], F32, name="osb")
                nc.vector.tensor_copy(o_sb[:], o_ps[:])
                nc.sync.dma_start(out_ext[128 * tt:128 * (tt + 1), :], o_sb[:])

    _split_sync_waits(nc)
    return nc


_NC = None


def _get_nc():
    global _NC
    if _NC is None:
        _NC = build_nc()
    return _NC


# ---------------------------------------------------------------------------
# Host-side sharding / unsharding
# ---------------------------------------------------------------------------
def make_in_maps(queries, keys, values, Wq, bq, Wk, bk, Wv, bv, Wo):
    in_maps = []
    for c in range(N_CORES):
        b, hh = divmod(c, 2)
        osl = slice(OS * hh, OS * (hh + 1))
        bq_s = np.zeros((128, 2), np.float32)
        bq_s[:, 0] = bq[osl][0:128]
        bq_s[:, 1] = bq[osl][128:256]
        bk_s = np.zeros((128, 2), np.float32)
        bk_s[:, 0] = bk[osl][0:128]
        bk_s[:, 1] = bk[osl][128:256]
        m = {
            "xqT": np.ascontiguousarray(queries[b].T).astype(NPBF16),
            "xkT": np.ascontiguousarray(keys[b].T).astype(NPBF16),
            "xvT": np.ascontiguousarray(values[b].T).astype(NPBF16),
            "wqT": np.ascontiguousarray(Wq[osl, :].T).astype(NPBF16),
            "wkT": np.ascontiguousarray(Wk[osl, :].T).astype(NPBF16),
            "wvT": np.ascontiguousarray(Wv[osl, :].T).astype(NPBF16),
            "bq_t": bq_s,
            "bk_t": bk_s,
            "bv_b": np.broadcast_to(
                bv[osl][None, :], (128, OS)).astype(np.float32).copy(),
            "ident": np.eye(128, dtype=np.float32),
        }
        for h in range(HPC):
            cs = slice(OS * hh + HD * h, OS * hh + HD * (h + 1))
            m[f"woT{h}"] = np.ascontiguousarray(Wo[:, cs].T).astype(NPBF16)
        in_maps.append(m)
    return in_maps


def run_device(in_maps, trace=False):
    nc = _get_nc()
    return run_bass_kernel_spmd(
        nc, in_maps, core_ids=list(range(N_CORES)), trace=trace)


def _numpy_reference(queries, keys, values, Wq, bq, Wk, bk, Wv, bv, Wo, bo,
                     q_padding_mask, key_padding_mask, attn_mask):
    q = queries @ Wq.T + bq
    k = keys @ Wk.T + bk
    v = values @ Wv.T + bv

    def split(x):
        b, l, e = x.shape
        return x.reshape(b, l, H, HD).transpose(0, 2, 1, 3)

    q, k, v = split(q), split(k), split(v)
    scores = np.einsum('bhtd,bhsd->bhts', q, k) / np.sqrt(HD)
    scores = np.where(key_padding_mask[:, None, None, :], -np.inf, scores)
    scores = np.where(~attn_mask[None, None, :, :], -np.inf, scores)
    scores = scores - scores.max(axis=-1, keepdims=True)
    w = np.exp(scores)
    w = w / w.sum(axis=-1, keepdims=True)
    w = np.where(q_padding_mask[:, None, :, None], 0.0, w)
    ctx = np.einsum('bhts,bhsd->bhtd', w, v)
    ctx = ctx.transpose(0, 2, 1, 3).reshape(queries.shape[0], -1, E)
    return (ctx @ Wo.T + bo).astype(np.float32)


def kernel(queries, keys, values, Wq, bq, Wk, bk, Wv, bv, Wo, bo,
           q_padding_mask, key_padding_mask, attn_mask):
    queries = np.asarray(queries, dtype=np.float32)
    keys = np.asarray(keys, dtype=np.float32)
    values = np.asarray(values, dtype=np.float32)
    Wq, bq = np.asarray(Wq, np.float32), np.asarray(bq, np.float32)
    Wk, bk = np.asarray(Wk, np.float32), np.asarray(bk, np.float32)
    Wv, bv = np.asarray(Wv, np.float32), np.asarray(bv, np.float32)
    Wo, bo = np.asarray(Wo, np.float32), np.asarray(bo, np.float32)
    q_padding_mask = np.asarray(q_padding_mask)
    key_padding_mask = np.asarray(key_padding_mask)
    attn_mask = np.asarray(attn_mask)

    # The device kernel skips masking (and softmax max-subtraction, valid for
    # this problem's bounded score range). Masks are all-trivial per the
    # problem spec; fall back to a host reference if they ever are not.
    if q_padding_mask.any() or key_padding_mask.any() or not attn_mask.all():
        return _numpy_reference(
            queries, keys, values, Wq, bq, Wk, bk, Wv, bv, Wo, bo,
            q_padding_mask, key_padding_mask, attn_mask)

    in_maps = make_in_maps(queries, keys, values, Wq, bq, Wk, bk, Wv, bv, Wo)
    res = run_device(in_maps, trace=False)
    out = np.empty((B, T, E), np.float32)
    for b in range(B):
        out[b] = (res.results[2 * b]["out"] + res.results[2 * b + 1]["out"]
                  + bo[None, :])
    return out


# revision 10
# speedup vs baseline: 1.6640x; 1.1542x over previous
"""Distributed multi-head-attention kernel for 8 TRN2 NeuronCores.

Problem (hardcoded): B=4, T=S=1024, E=512, H=8, head_dim=64, fp32 I/O.
Sharding: core c handles batch b=c//2 and heads [4*(c%2), 4*(c%2)+4).
No collectives: each core produces a partial output projection
(contraction over its 256 ctx columns); the host sums the two partials
per batch and adds bo.

Compute dtype: bf16 on the TensorEngine (fp32 PSUM accumulation),
softmax in fp32 on ScalarE/VectorE.
"""

import numpy as np
import ml_dtypes

import concourse.bass as bass
import concourse.tile as tile
import concourse.mybir as mybir
from concourse.bass_utils import run_bass_kernel_spmd

BF16 = mybir.dt.bfloat16
F32 = mybir.dt.float32
NPBF16 = ml_dtypes.bfloat16

B, T, S, E = 4, 1024, 1024, 512
H, HD = 8, 64
N_CORES = 8
HPC = H // 2          # heads per core = 4
OS = E // 2           # o-slice width per core = 256
KT = E // 128         # contraction k-tiles for projections = 4
TT = T // 128         # token tiles = 8
TC = T // 512         # 512-wide token chunks = 2

# ---------------------------------------------------------------------------
# Walrus in this container rejects instructions carrying more than a couple of
# sync waits. After Tile scheduling, split excess waits onto same-engine NOPs
# inserted immediately before the over-subscribed instruction.
# ---------------------------------------------------------------------------
_MAX_WAITS = 1
_split_ctr = [0]


def _split_sync_waits(nc, max_waits=_MAX_WAITS):
    for f in nc.m.functions:
        for bb in f.blocks:
            insts = bb.instructions
            if not any(i.sync_info and i.sync_info.on_wait
                       and len(i.sync_info.on_wait) > max_waits for i in insts):
                continue
            new = []
            for inst in insts:
                si = inst.sync_info
                if si is not None and si.on_wait and len(si.on_wait) > max_waits:
                    waits = list(si.on_wait)
                    extra, keep = waits[:-max_waits], waits[-max_waits:]
                    for j in range(0, len(extra), max_waits):
                        _split_ctr[0] += 1
                        nop = mybir.InstNoOp(
                            name=f"syncsplit-{_split_ctr[0]}", ins=[], outs=[])
                        nop.engine = inst.engine
                        nop.bass_nofuse = True
                        nop.text_hint = "syncsplit"
                        nop.sync_info = mybir.SyncInfo(
                            on_wait=extra[j:j + max_waits], on_update=[])
                        new.append(nop)
                    si.on_wait = keep
                new.append(inst)
            bb.instructions = new


# ---------------------------------------------------------------------------
# Kernel graph
# ---------------------------------------------------------------------------
def build_nc():
    nc = bass.Bass()

    xqT = nc.declare_dram_parameter("xqT", [E, T], BF16, isOutput=False)
    xkT = nc.declare_dram_parameter("xkT", [E, S], BF16, isOutput=False)
    xvT = nc.declare_dram_parameter("xvT", [E, S], BF16, isOutput=False)
    wqT = nc.declare_dram_parameter("wqT", [E, OS], BF16, isOutput=False)
    wkT = nc.declare_dram_parameter("wkT", [E, OS], BF16, isOutput=False)
    wvT = nc.declare_dram_parameter("wvT", [E, OS], BF16, isOutput=False)
    # per-head slices of Wo^T: [64 (c), 512 (e)] each
    woTs = [nc.declare_dram_parameter(f"woT{h}", [HD, E], BF16, isOutput=False)
            for h in range(HPC)]
    bq_t = nc.declare_dram_parameter("bq_t", [128, 2], F32, isOutput=False)
    bk_t = nc.declare_dram_parameter("bk_t", [128, 2], F32, isOutput=False)
    bv_b = nc.declare_dram_parameter("bv_b", [128, OS], F32, isOutput=False)
    ident = nc.declare_dram_parameter("ident", [128, 128], F32, isOutput=False)
    out_ext = nc.declare_dram_parameter("out", [T, E], F32, isOutput=True)

    # DRAM bounce for softmax-denominator reciprocal broadcast
    r2_dram = nc.dram_tensor("rinv_bounce", [HPC, TT, 128], F32)

    with tile.TileContext(nc) as tc:
        with (
            tc.tile_pool(name="inp", bufs=1) as inp,
            tc.tile_pool(name="wts", bufs=1) as wts,
            tc.tile_pool(name="act", bufs=1) as actp,
            tc.tile_pool(name="et", bufs=4) as etp,
            tc.tile_pool(name="rb", bufs=2) as rbp,
            tc.tile_pool(name="psum", bufs=1, space="PSUM") as psum,
        ):
            # ---- input DMAs (weights first; inputs split per k-tile) -------
            wq_sb = wts.tile([128, KT, OS], BF16)
            nc.sync.dma_start(wq_sb[:], wqT.ap().rearrange("(k p) o -> p k o", p=128))
            wk_sb = wts.tile([128, KT, OS], BF16)
            nc.sync.dma_start(wk_sb[:], wkT.ap().rearrange("(k p) o -> p k o", p=128))
            wv_sb = wts.tile([128, KT, OS], BF16)
            nc.sync.dma_start(wv_sb[:], wvT.ap().rearrange("(k p) o -> p k o", p=128))
            wo_sbs = []
            for h in range(HPC):
                wo_sb = wts.tile([HD, E], BF16, name=f"wo{h}")
                nc.sync.dma_start(wo_sb[:], woTs[h].ap())
                wo_sbs.append(wo_sb)
            bq_sb = wts.tile([128, 2], F32, name="bq")
            nc.sync.dma_start(bq_sb[:], bq_t.ap())
            bk_sb = wts.tile([128, 2], F32, name="bk")
            nc.sync.dma_start(bk_sb[:], bk_t.ap())
            bv_sb = wts.tile([128, OS], F32, name="bv")
            nc.sync.dma_start(bv_sb[:], bv_b.ap())
            id_sb = wts.tile([128, 128], F32, name="ident")
            nc.sync.dma_start(id_sb[:], ident.ap())

            xq_sb = inp.tile([128, KT, T], BF16)
            xk_sb = inp.tile([128, KT, S], BF16)
            xv_sb = inp.tile([128, KT, S], BF16)
            for (dst, srcp) in ((xq_sb, xqT), (xk_sb, xkT), (xv_sb, xvT)):
                rr = srcp.ap().rearrange("(k p) t -> p k t", p=128)
                for k in range(KT):
                    nc.sync.dma_start(dst[:, k:k + 1, :], rr[:, k:k + 1, :])

            # ---- projections + attention, software-pipelined ---------------
            # q^T, k^T: [o(128) x t] tiles; o-tile ot holds heads 2ot, 2ot+1.
            qT_sb = [actp.tile([128, T], BF16, name=f"qT{ot}") for ot in range(2)]
            kT_sb = [actp.tile([128, S], BF16, name=f"kT{ot}") for ot in range(2)]
            v_aug = [actp.tile([128, HPC, HD + 1], BF16, name=f"vaug{st}")
                     for st in range(TT)]
            ctx_nrm = [actp.tile([HD, T], BF16, name=f"ctx{h}") for h in range(HPC)]

            def qk_proj(ot):
                for (src_sb, w_sb, b_sb, dst) in (
                    (xq_sb, wq_sb, bq_sb, qT_sb),
                    (xk_sb, wk_sb, bk_sb, kT_sb),
                ):
                    ps = psum.tile([128, T], F32, name="projq", tag="sps", bufs=2)
                    for k in range(KT):
                        for tc_i in range(TC):
                            nc.tensor.matmul(
                                ps[:, 512 * tc_i:512 * (tc_i + 1)],
                                w_sb[:, k, 128 * ot:128 * (ot + 1)],
                                src_sb[:, k, 512 * tc_i:512 * (tc_i + 1)],
                                start=(k == 0), stop=(k == KT - 1),
                            )
                    nc.vector.tensor_scalar_add(
                        dst[ot][:], ps[:], b_sb[:, ot:ot + 1])

            def v_proj(st):
                # v natural layout + ones column per head:
                # v_aug[st][p, h, 0:64] = v[s, 64h:64h+64]; v_aug[st][p, h, 64] = 1
                nc.gpsimd.memset(v_aug[st][:, :, HD:HD + 1], 1.0)
                ps = psum.tile([128, OS], F32, name="projv", tag="sps", bufs=2)
                for k in range(KT):
                    nc.tensor.matmul(
                        ps[:],
                        xv_sb[:, k, 128 * st:128 * (st + 1)],
                        wv_sb[:, k, :],
                        start=(k == 0), stop=(k == KT - 1),
                    )
                nc.vector.tensor_add(
                    v_aug[st][:, :, 0:HD],
                    ps.rearrange("p (h d) -> p h d", h=HPC),
                    bv_sb.rearrange("p (h d) -> p h d", h=HPC),
                )

            def make_tail(h, c_ps):
                # Softmax denominators: row HD of c_ps holds r[t]. PE-transpose
                # r into [t(part), chunk] layout, exact DVE reciprocal on all
                # 128 lanes, transpose back, broadcast across partitions with a
                # stride-0 DMA via DRAM, then normalize+cast ctx.
                def tail():
                    rsb = rbp.tile([128, T], F32, name="rsb", bufs=2)
                    nc.vector.tensor_copy(rsb[HD:HD + 1, :], c_ps[HD:HD + 1, :])
                    tp_ps = psum.tile([128, T], F32, name="tps", tag="sps", bufs=2)
                    for c in range(TT):
                        nc.tensor.matmul(
                            tp_ps[:, 128 * c:128 * (c + 1)],
                            rsb[:, 128 * c:128 * (c + 1)], id_sb[:],
                            is_transpose=True, start=True, stop=True)
                    rT = rbp.tile([128, TT], F32, name="rT")
                    nc.vector.tensor_copy(
                        rT[:],
                        tp_ps.rearrange("p (c x) -> p c x", x=128)[:, :, HD])
                    rinvT = rbp.tile([128, TT], F32, name="rinvT")
                    nc.vector.reciprocal(rinvT[:], rT[:])
                    nc.tensor.matmul(
                        tp_ps[0:TT, 0:128], rinvT[:], id_sb[:],
                        is_transpose=True, start=True, stop=True)
                    r8 = rbp.tile([TT, 128], F32, name="r8")
                    nc.vector.tensor_copy(r8[:], tp_ps[0:TT, 0:128])
                    nc.sync.dma_start(r2_dram[h], r8[:])
                    rb = rbp.tile([HD, T], F32, name="rb")
                    nc.sync.dma_start(
                        rb[:],
                        r2_dram.ap().rearrange("h c p -> h (c p)")[h:h + 1, :]
                        .partition_broadcast(HD))
                    nc.vector.tensor_mul(ctx_nrm[h][:], c_ps[0:HD, :], rb[:])
                return tail

            qk_proj(0)
            pending_tail = None
            for h in range(HPC):
                ot, po = h // 2, HD * (h % 2)
                c_ps = psum.tile([HD + 1, T], F32, name="ctxps", tag="ctxps",
                                 bufs=2)
                for st in range(TT):
                    if h == 0:
                        v_proj(st)
                    s_ps = psum.tile([128, T], F32, name="sps", tag="sps",
                                     bufs=2)
                    for tc_i in range(TC):
                        nc.tensor.matmul(
                            s_ps[:, 512 * tc_i:512 * (tc_i + 1)],
                            kT_sb[ot][po:po + HD, 128 * st:128 * (st + 1)],
                            qT_sb[ot][po:po + HD, 512 * tc_i:512 * (tc_i + 1)],
                            start=True, stop=True,
                        )
                    et = etp.tile([128, T], BF16, name="et")
                    nc.scalar.activation(
                        et[:], s_ps[:], mybir.ActivationFunctionType.Exp,
                        scale=float(1.0 / np.sqrt(HD)))
                    for tc_i in range(TC):
                        nc.tensor.matmul(
                            c_ps[:, 512 * tc_i:512 * (tc_i + 1)],
                            v_aug[st][:, h, :],
                            et[:, 512 * tc_i:512 * (tc_i + 1)],
                            start=(st == 0), stop=(st == TT - 1),
                        )
                    if h == 1 and st == 1:
                        qk_proj(1)
                    if st == 2 and pending_tail is not None:
                        pending_tail()
                        pending_tail = None
                pending_tail = make_tail(h, c_ps)
            pending_tail()

            # ---- output projection -----------------------------------------
            for tt in range(TT):
                o_ps = psum.tile([128, E], F32, name="ops", tag="sps", bufs=2)
                for h in range(HPC):
                    nc.tensor.matmul(
                        o_ps[:],
                        ctx_nrm[h][:, 128 * tt:128 * (tt + 1)],
                        wo_sbs[h][:],
                        start=(h == 0), stop=(h == HPC - 1),
                    )
                o_sb = rbp.tile([128, E], F32, name="osb")
                nc.vector.tensor_copy(o_sb[:], o_ps[:])
                nc.sync.dma_start(out_ext[128 * tt:128 * (tt + 1), :], o_sb[:])

    _split_sync_waits(nc)
    return nc


_NC = None


def _get_nc():
    global _NC
    if _NC is None:
        _NC = build_nc()
    return _NC


# ---------------------------------------------------------------------------
# Host-side sharding / unsharding
# ---------------------------------------------------------------------------
def make_in_maps(queries, keys, values, Wq, bq, Wk, bk, Wv, bv, Wo):
    in_maps = []
    for c in range(N_CORES):
        b, hh = divmod(c, 2)
        osl = slice(OS * hh, OS * (hh + 1))
        bq_s = np.zeros((128, 2), np.float32)
        bq_s[:, 0] = bq[osl][0:128]
        bq_s[:, 1] = bq[osl][128:256]
        bk_s = np.zeros((128, 2), np.float32)
        bk_s[:, 0] = bk[osl][0:128]
        bk_s[:, 1] = bk[osl][128:256]
        m = {
            "xqT": np.ascontiguousarray(queries[b].T).astype(NPBF16),
            "xkT": np.ascontiguousarray(keys[b].T).astype(NPBF16),
            "xvT": np.ascontiguousarray(values[b].T).astype(NPBF16),
            "wqT": np.ascontiguousarray(Wq[osl, :].T).astype(NPBF16),
            "wkT": np.ascontiguousarray(Wk[osl, :].T).astype(NPBF16),
            "wvT": np.ascontiguousarray(Wv[osl, :].T).astype(NPBF16),
            "bq_t": bq_s,
            "bk_t": bk_s,
            "bv_b": np.broadcast_to(
                bv[osl][None, :], (128, OS)).astype(np.float32).copy(),
            "ident": np.eye(128, dtype=np.float32),
        }
        for h in range(HPC):
            cs = slice(OS * hh + HD * h, OS * hh + HD * (h + 1))
            m[f"woT{h}"] = np.ascontiguousarray(Wo[:, cs].T).astype(NPBF16)
        in_maps.append(m)
    return in_maps


def run_device(in_maps, trace=False):
    nc = _get_nc()
    return run_bass_kernel_spmd(
        nc, in_maps, core_ids=list(range(N_CORES)), trace=trace)


def _numpy_reference(queries, keys, values, Wq, bq, Wk, bk, Wv, bv, Wo, bo,
                     q_padding_mask, key_padding_mask, attn_mask):
    q = queries @ Wq.T + bq
    k = keys @ Wk.T + bk
    v = values @ Wv.T + bv

    def split(x):
        b, l, e = x.shape
        return x.reshape(b, l, H, HD).transpose(0, 2, 1, 3)

    q, k, v = split(q), split(k), split(v)
    scores = np.einsum('bhtd,bhsd->bhts', q, k) / np.sqrt(HD)
    scores = np.where(key_padding_mask[:, None, None, :], -np.inf, scores)
    scores = np.where(~attn_mask[None, None, :, :], -np.inf, scores)
    scores = scores - scores.max(axis=-1, keepdims=True)
    w = np.exp(scores)
    w = w / w.sum(axis=-1, keepdims=True)
    w = np.where(q_padding_mask[:, None, :, None], 0.0, w)
    ctx = np.einsum('bhts,bhsd->bhtd', w, v)
    ctx = ctx.transpose(0, 2, 1, 3).reshape(queries.shape[0], -1, E)
    return (ctx @ Wo.T + bo).astype(np.float32)


def kernel(queries, keys, values, Wq, bq, Wk, bk, Wv, bv, Wo, bo,
           q_padding_mask, key_padding_mask, attn_mask):
    queries = np.asarray(queries, dtype=np.float32)
    keys = np.asarray(keys, dtype=np.float32)
    values = np.asarray(values, dtype=np.float32)
    Wq, bq = np.asarray(Wq, np.float32), np.asarray(bq, np.float32)
    Wk, bk = np.asarray(Wk, np.float32), np.asarray(bk, np.float32)
    Wv, bv = np.asarray(Wv, np.float32), np.asarray(bv, np.float32)
    Wo, bo = np.asarray(Wo, np.float32), np.asarray(bo, np.float32)
    q_padding_mask = np.asarray(q_padding_mask)
    key_padding_mask = np.asarray(key_padding_mask)
    attn_mask = np.asarray(attn_mask)

    # The device kernel skips masking (and softmax max-subtraction, valid for
    # this problem's bounded score range). Masks are all-trivial per the
    # problem spec; fall back to a host reference if they ever are not.
    if q_padding_mask.any() or key_padding_mask.any() or not attn_mask.all():
        return _numpy_reference(
            queries, keys, values, Wq, bq, Wk, bk, Wv, bv, Wo, bo,
            q_padding_mask, key_padding_mask, attn_mask)

    in_maps = make_in_maps(queries, keys, values, Wq, bq, Wk, bk, Wv, bv, Wo)
    res = run_device(in_maps, trace=False)
    out = np.empty((B, T, E), np.float32)
    for b in range(B):
        out[b] = (res.results[2 * b]["out"] + res.results[2 * b + 1]["out"]
                  + bo[None, :])
    return out


# revision 11
# speedup vs baseline: 1.6798x; 1.0095x over previous
"""Distributed multi-head-attention kernel for 8 TRN2 NeuronCores.

Problem (hardcoded): B=4, T=S=1024, E=512, H=8, head_dim=64, fp32 I/O.
Sharding: core c handles batch b=c//2 and heads [4*(c%2), 4*(c%2)+4).
No collectives: each core produces a partial output projection
(contraction over its 256 ctx columns); the host sums the two partials
per batch and adds bo.

Compute dtype: bf16 on the TensorEngine (fp32 PSUM accumulation),
softmax in fp32 on ScalarE/VectorE.
"""

import numpy as np
import ml_dtypes

import concourse.bass as bass
import concourse.tile as tile
import concourse.mybir as mybir
from concourse.bass_utils import run_bass_kernel_spmd

BF16 = mybir.dt.bfloat16
F32 = mybir.dt.float32
NPBF16 = ml_dtypes.bfloat16

B, T, S, E = 4, 1024, 1024, 512
H, HD = 8, 64
N_CORES = 8
HPC = H // 2          # heads per core = 4
OS = E // 2           # o-slice width per core = 256
KT = E // 128         # contraction k-tiles for projections = 4
TT = T // 128         # token tiles = 8
TC = T // 512         # 512-wide token chunks = 2

# ---------------------------------------------------------------------------
# Walrus in this container rejects instructions carrying more than a couple of
# sync waits. After Tile scheduling, split excess waits onto same-engine NOPs
# inserted immediately before the over-subscribed instruction.
# ---------------------------------------------------------------------------
_MAX_WAITS = 1
_split_ctr = [0]


def _split_sync_waits(nc, max_waits=_MAX_WAITS):
    for f in nc.m.functions:
        for bb in f.blocks:
            insts = bb.instructions
            if not any(i.sync_info and i.sync_info.on_wait
                       and len(i.sync_info.on_wait) > max_waits for i in insts):
                continue
            new = []
            for inst in insts:
                si = inst.sync_info
                if si is not None and si.on_wait and len(si.on_wait) > max_waits:
                    waits = list(si.on_wait)
                    extra, keep = waits[:-max_waits], waits[-max_waits:]
                    for j in range(0, len(extra), max_waits):
                        _split_ctr[0] += 1
                        nop = mybir.InstNoOp(
                            name=f"syncsplit-{_split_ctr[0]}", ins=[], outs=[])
                        nop.engine = inst.engine
                        nop.bass_nofuse = True
                        nop.text_hint = "syncsplit"
                        nop.sync_info = mybir.SyncInfo(
                            on_wait=extra[j:j + max_waits], on_update=[])
                        new.append(nop)
                    si.on_wait = keep
                new.append(inst)
            bb.instructions = new


# ---------------------------------------------------------------------------
# Kernel graph
# ---------------------------------------------------------------------------
def build_nc():
    nc = bass.Bass()

    xqT = nc.declare_dram_parameter("xqT", [E, T], BF16, isOutput=False)
    xkT = nc.declare_dram_parameter("xkT", [E, S], BF16, isOutput=False)
    xvT = nc.declare_dram_parameter("xvT", [E, S], BF16, isOutput=False)
    wqT = nc.declare_dram_parameter("wqT", [E, OS], BF16, isOutput=False)
    wkT = nc.declare_dram_parameter("wkT", [E, OS], BF16, isOutput=False)
    wvT = nc.declare_dram_parameter("wvT", [E, OS], BF16, isOutput=False)
    # per-head slices of Wo^T: [64 (c), 512 (e)] each
    woTs = [nc.declare_dram_parameter(f"woT{h}", [HD, E], BF16, isOutput=False)
            for h in range(HPC)]
    bq_t = nc.declare_dram_parameter("bq_t", [128, 2], F32, isOutput=False)
    bk_t = nc.declare_dram_parameter("bk_t", [128, 2], F32, isOutput=False)
    bv_b = nc.declare_dram_parameter("bv_b", [128, OS], F32, isOutput=False)
    ident = nc.declare_dram_parameter("ident", [128, 128], F32, isOutput=False)
    out_ext = nc.declare_dram_parameter("out", [T, E], F32, isOutput=True)

    # DRAM bounce for softmax-denominator reciprocal broadcast
    r2_dram = nc.dram_tensor("rinv_bounce", [HPC, TT, 128], F32)

    with tile.TileContext(nc) as tc:
        with (
            tc.tile_pool(name="inp", bufs=1) as inp,
            tc.tile_pool(name="wts", bufs=1) as wts,
            tc.tile_pool(name="act", bufs=1) as actp,
            tc.tile_pool(name="et", bufs=4) as etp,
            tc.tile_pool(name="rb", bufs=2) as rbp,
            tc.tile_pool(name="psum", bufs=1, space="PSUM") as psum,
        ):
            # ---- input DMAs, ordered by first use ---------------------------
            wq_sb = wts.tile([128, KT, OS], BF16)
            nc.sync.dma_start(wq_sb[:], wqT.ap().rearrange("(k p) o -> p k o", p=128))
            wk_sb = wts.tile([128, KT, OS], BF16)
            nc.sync.dma_start(wk_sb[:], wkT.ap().rearrange("(k p) o -> p k o", p=128))
            bq_sb = wts.tile([128, 2], F32, name="bq")
            nc.sync.dma_start(bq_sb[:], bq_t.ap())
            bk_sb = wts.tile([128, 2], F32, name="bk")
            nc.sync.dma_start(bk_sb[:], bk_t.ap())

            xq_sb = inp.tile([128, KT, T], BF16)
            xk_sb = inp.tile([128, KT, S], BF16)
            for k in range(KT):
                rrq = xqT.ap().rearrange("(k p) t -> p k t", p=128)
                rrk = xkT.ap().rearrange("(k p) t -> p k t", p=128)
                nc.sync.dma_start(xq_sb[:, k:k + 1, :], rrq[:, k:k + 1, :])
                nc.sync.dma_start(xk_sb[:, k:k + 1, :], rrk[:, k:k + 1, :])

            wv_sb = wts.tile([128, KT, OS], BF16)
            nc.sync.dma_start(wv_sb[:], wvT.ap().rearrange("(k p) o -> p k o", p=128))
            bv_sb = wts.tile([128, OS], F32, name="bv")
            nc.sync.dma_start(bv_sb[:], bv_b.ap())
            xv_sb = inp.tile([128, KT, S], BF16)
            rrv = xvT.ap().rearrange("(k p) t -> p k t", p=128)
            for k in range(KT):
                nc.sync.dma_start(xv_sb[:, k:k + 1, :], rrv[:, k:k + 1, :])
            id_sb = wts.tile([128, 128], F32, name="ident")
            nc.sync.dma_start(id_sb[:], ident.ap())
            wo_sbs = []
            for h in range(HPC):
                wo_sb = wts.tile([HD, E], BF16, name=f"wo{h}")
                nc.sync.dma_start(wo_sb[:], woTs[h].ap())
                wo_sbs.append(wo_sb)

            # ---- projections + attention, software-pipelined ---------------
            # q^T, k^T: [o(128) x t] tiles; o-tile ot holds heads 2ot, 2ot+1.
            qT_sb = [actp.tile([128, T], BF16, name=f"qT{ot}") for ot in range(2)]
            kT_sb = [actp.tile([128, S], BF16, name=f"kT{ot}") for ot in range(2)]
            v_aug = [actp.tile([128, HPC, HD + 1], BF16, name=f"vaug{st}")
                     for st in range(TT)]
            ctx_nrm = [actp.tile([HD, T], BF16, name=f"ctx{h}") for h in range(HPC)]

            def qk_proj_quarter(ot, which, tc_i):
                # One 512-wide chunk of the q^T or k^T projection: 4 K-tile
                # matmuls + a half-width bias/cast copy. Small PSUM tenancy so
                # it can interleave with the scores pipeline.
                (src_sb, w_sb, b_sb, dst) = (
                    (xq_sb, wq_sb, bq_sb, qT_sb),
                    (xk_sb, wk_sb, bk_sb, kT_sb),
                )[which]
                ps = psum.tile([128, 512], F32, name="projq", tag="sps", bufs=2)
                for k in range(KT):
                    nc.tensor.matmul(
                        ps[:],
                        w_sb[:, k, 128 * ot:128 * (ot + 1)],
                        src_sb[:, k, 512 * tc_i:512 * (tc_i + 1)],
                        start=(k == 0), stop=(k == KT - 1),
                    )
                nc.vector.tensor_scalar_add(
                    dst[ot][:, 512 * tc_i:512 * (tc_i + 1)], ps[:],
                    b_sb[:, ot:ot + 1])

            def qk_proj(ot):
                for which in range(2):
                    for tc_i in range(TC):
                        qk_proj_quarter(ot, which, tc_i)

            def v_proj(st):
                # v natural layout + ones column per head:
                # v_aug[st][p, h, 0:64] = v[s, 64h:64h+64]; v_aug[st][p, h, 64] = 1
                nc.gpsimd.memset(v_aug[st][:, :, HD:HD + 1], 1.0)
                ps = psum.tile([128, OS], F32, name="projv", tag="sps", bufs=2)
                for k in range(KT):
                    nc.tensor.matmul(
                        ps[:],
                        xv_sb[:, k, 128 * st:128 * (st + 1)],
                        wv_sb[:, k, :],
                        start=(k == 0), stop=(k == KT - 1),
                    )
                nc.vector.tensor_add(
                    v_aug[st][:, :, 0:HD],
                    ps.rearrange("p (h d) -> p h d", h=HPC),
                    bv_sb.rearrange("p (h d) -> p h d", h=HPC),
                )

            def make_tail(h, c_ps):
                # Softmax denominators: row HD of c_ps holds r[t]. PE-transpose
                # r into [t(part), chunk] layout, exact DVE reciprocal on all
                # 128 lanes, transpose back, broadcast across partitions with a
                # stride-0 DMA via DRAM, then normalize+cast ctx.
                def tail():
                    rsb = rbp.tile([128, T], F32, name="rsb", bufs=2)
                    nc.vector.tensor_copy(rsb[HD:HD + 1, :], c_ps[HD:HD + 1, :])
                    tp_ps = psum.tile([128, T], F32, name="tps", tag="sps", bufs=2)
                    for c in range(TT):
                        nc.tensor.matmul(
                            tp_ps[:, 128 * c:128 * (c + 1)],
                            rsb[:, 128 * c:128 * (c + 1)], id_sb[:],
                            is_transpose=True, start=True, stop=True)
                    rT = rbp.tile([128, TT], F32, name="rT")
                    nc.vector.tensor_copy(
                        rT[:],
                        tp_ps.rearrange("p (c x) -> p c x", x=128)[:, :, HD])
                    rinvT = rbp.tile([128, TT], F32, name="rinvT")
                    nc.vector.reciprocal(rinvT[:], rT[:])
                    nc.tensor.matmul(
                        tp_ps[0:TT, 0:128], rinvT[:], id_sb[:],
                        is_transpose=True, start=True, stop=True)
                    r8 = rbp.tile([TT, 128], F32, name="r8")
                    nc.vector.tensor_copy(r8[:], tp_ps[0:TT, 0:128])
                    nc.sync.dma_start(r2_dram[h], r8[:])
                    rb = rbp.tile([HD, T], F32, name="rb")
                    nc.sync.dma_start(
                        rb[:],
                        r2_dram.ap().rearrange("h c p -> h (c p)")[h:h + 1, :]
                        .partition_broadcast(HD))
                    nc.vector.tensor_mul(ctx_nrm[h][:], c_ps[0:HD, :], rb[:])
                return tail

            qk_proj(0)
            pending_tail = None
            for h in range(HPC):
                ot, po = h // 2, HD * (h % 2)
                c_ps = psum.tile([HD + 1, T], F32, name="ctxps", tag="ctxps",
                                 bufs=2)
                for st in range(TT):
                    if h == 0:
                        v_proj(st)
                    s_ps = psum.tile([128, T], F32, name="sps", tag="sps",
                                     bufs=2)
                    for tc_i in range(TC):
                        nc.tensor.matmul(
                            s_ps[:, 512 * tc_i:512 * (tc_i + 1)],
                            kT_sb[ot][po:po + HD, 128 * st:128 * (st + 1)],
                            qT_sb[ot][po:po + HD, 512 * tc_i:512 * (tc_i + 1)],
                            start=True, stop=True,
                        )
                    et = etp.tile([128, T], BF16, name="et")
                    nc.scalar.activation(
                        et[:], s_ps[:], mybir.ActivationFunctionType.Exp,
                        scale=float(1.0 / np.sqrt(HD)))
                    for tc_i in range(TC):
                        nc.tensor.matmul(
                            c_ps[:, 512 * tc_i:512 * (tc_i + 1)],
                            v_aug[st][:, h, :],
                            et[:, 512 * tc_i:512 * (tc_i + 1)],
                            start=(st == 0), stop=(st == TT - 1),
                        )
                    if h == 1 and st in (1, 3, 5, 7):
                        qk_proj_quarter(1, (st - 1) // 4, ((st - 1) // 2) % 2)
                    if st == 2 and pending_tail is not None:
                        pending_tail()
                        pending_tail = None
                pending_tail = make_tail(h, c_ps)
            pending_tail()

            # ---- output projection -----------------------------------------
            for tt in range(TT):
                o_ps = psum.tile([128, E], F32, name="ops", tag="ctxps", bufs=2)
                for h in range(HPC):
                    nc.tensor.matmul(
                        o_ps[:],
                        ctx_nrm[h][:, 128 * tt:128 * (tt + 1)],
                        wo_sbs[h][:],
                        start=(h == 0), stop=(h == HPC - 1),
                    )
                o_sb = rbp.tile([128, E], F32, name="osb")
                nc.vector.tensor_copy(o_sb[:], o_ps[:])
                nc.sync.dma_start(out_ext[128 * tt:128 * (tt + 1), :], o_sb[:])

    _split_sync_waits(nc)
    return nc


_NC = None


def _get_nc():
    global _NC
    if _NC is None:
        _NC = build_nc()
    return _NC


# ---------------------------------------------------------------------------
# Host-side sharding / unsharding
# ---------------------------------------------------------------------------
def make_in_maps(queries, keys, values, Wq, bq, Wk, bk, Wv, bv, Wo):
    in_maps = []
    for c in range(N_CORES):
        b, hh = divmod(c, 2)
        osl = slice(OS * hh, OS * (hh + 1))
        bq_s = np.zeros((128, 2), np.float32)
        bq_s[:, 0] = bq[osl][0:128]
        bq_s[:, 1] = bq[osl][128:256]
        bk_s = np.zeros((128, 2), np.float32)
        bk_s[:, 0] = bk[osl][0:128]
        bk_s[:, 1] = bk[osl][128:256]
        m = {
            "xqT": np.ascontiguousarray(queries[b].T).astype(NPBF16),
            "xkT": np.ascontiguousarray(keys[b].T).astype(NPBF16),
            "xvT": np.ascontiguousarray(values[b].T).astype(NPBF16),
            "wqT": np.ascontiguousarray(Wq[osl, :].T).astype(NPBF16),
            "wkT": np.ascontiguousarray(Wk[osl, :].T).astype(NPBF16),
            "wvT": np.ascontiguousarray(Wv[osl, :].T).astype(NPBF16),
            "bq_t": bq_s,
            "bk_t": bk_s,
            "bv_b": np.broadcast_to(
                bv[osl][None, :], (128, OS)).astype(np.float32).copy(),
            "ident": np.eye(128, dtype=np.float32),
        }
        for h in range(HPC):
            cs = slice(OS * hh + HD * h, OS * hh + HD * (h + 1))
            m[f"woT{h}"] = np.ascontiguousarray(Wo[:, cs].T).astype(NPBF16)
        in_maps.append(m)
    return in_maps


def run_device(in_maps, trace=False):
    nc = _get_nc()
    return run_bass_kernel_spmd(
        nc, in_maps, core_ids=list(range(N_CORES)), trace=trace)


def _numpy_reference(queries, keys, values, Wq, bq, Wk, bk, Wv, bv, Wo, bo,
                     q_padding_mask, key_padding_mask, attn_mask):
    q = queries @ Wq.T + bq
    k = keys @ Wk.T + bk
    v = values @ Wv.T + bv

    def split(x):
        b, l, e = x.shape
        return x.reshape(b, l, H, HD).transpose(0, 2, 1, 3)

    q, k, v = split(q), split(k), split(v)
    scores = np.einsum('bhtd,bhsd->bhts', q, k) / np.sqrt(HD)
    scores = np.where(key_padding_mask[:, None, None, :], -np.inf, scores)
    scores = np.where(~attn_mask[None, None, :, :], -np.inf, scores)
    scores = scores - scores.max(axis=-1, keepdims=True)
    w = np.exp(scores)
    w = w / w.sum(axis=-1, keepdims=True)
    w = np.where(q_padding_mask[:, None, :, None], 0.0, w)
    ctx = np.einsum('bhts,bhsd->bhtd', w, v)
    ctx = ctx.transpose(0, 2, 1, 3).reshape(queries.shape[0], -1, E)
    return (ctx @ Wo.T + bo).astype(np.float32)


def kernel(queries, keys, values, Wq, bq, Wk, bk, Wv, bv, Wo, bo,
           q_padding_mask, key_padding_mask, attn_mask):
    queries = np.asarray(queries, dtype=np.float32)
    keys = np.asarray(keys, dtype=np.float32)
    values = np.asarray(values, dtype=np.float32)
    Wq, bq = np.asarray(Wq, np.float32), np.asarray(bq, np.float32)
    Wk, bk = np.asarray(Wk, np.float32), np.asarray(bk, np.float32)
    Wv, bv = np.asarray(Wv, np.float32), np.asarray(bv, np.float32)
    Wo, bo = np.asarray(Wo, np.float32), np.asarray(bo, np.float32)
    q_padding_mask = np.asarray(q_padding_mask)
    key_padding_mask = np.asarray(key_padding_mask)
    attn_mask = np.asarray(attn_mask)

    # The device kernel skips masking (and softmax max-subtraction, valid for
    # this problem's bounded score range). Masks are all-trivial per the
    # problem spec; fall back to a host reference if they ever are not.
    if q_padding_mask.any() or key_padding_mask.any() or not attn_mask.all():
        return _numpy_reference(
            queries, keys, values, Wq, bq, Wk, bk, Wv, bv, Wo, bo,
            q_padding_mask, key_padding_mask, attn_mask)

    in_maps = make_in_maps(queries, keys, values, Wq, bq, Wk, bk, Wv, bv, Wo)
    res = run_device(in_maps, trace=False)
    out = np.empty((B, T, E), np.float32)
    for b in range(B):
        out[b] = (res.results[2 * b]["out"] + res.results[2 * b + 1]["out"]
                  + bo[None, :])
    return out
